# revision 6
# baseline (speedup 1.0000x reference)
"""ConditionalAttentionLayer (gnn_message_passing) Trainium2 kernel.

Sharding: one attention mechanism (head) per NeuronCore, 8 cores.
Each core computes its head's projection h_m = FiLM(x @ W_m), GAT scores,
masked softmax and out_m = attn_m @ h_m over all N=2708 nodes; the host
concatenates the 8 [N, 64] slices into [N, 512].

Math trick used on-device: with s_ij = es_i + ed_j,
  exp(leakyrelu(s)) = max(exp(s), exp(0.2 s))          (exp is monotone)
and softmax over j is invariant to any per-i scale, so dividing by
exp(es_i) gives
  E_ij = adj_ji * max(G_i * D_j, B_j)
with G = exp(-0.8 es), B = exp(ed), D = exp(0.2 ed).  Each [128, 1536]
attention tile is then just one dual-op tensor_scalar (mult+max) and one
masked multiply on DVE — no per-tile transcendentals.  Row sums for the
softmax come free from a ones-column appended to the value matrix in the
PE matmul (output is computed transposed, [65, i], then PE-transposed
back and normalized).
"""

import numpy as np
import ml_dtypes

N = 2708
INS = 1433
OUTS = 64
M = 8
HID = 64

FCH = 12            # feature chunks of 128 (1433 + ones row -> 1536)
FPAD = FCH * 128    # 1536
NB = 22             # node blocks of 128
NPAD = NB * 128     # 2816
NGRP = 2            # i groups
IG = 1536           # i group width
IPAD = NGRP * IG    # 3072
NSUB = 3            # 512-wide sub-matmuls per i group

_CACHE = {}


def _build_nc():
    import concourse.bass as bass
    import concourse.mybir as mybir
    import concourse.tile as tile
    from concourse.masks import make_identity

    f32 = mybir.dt.float32
    bf16 = mybir.dt.bfloat16
    Alu = mybir.AluOpType
    Act = mybir.ActivationFunctionType

    nc = bass.Bass("TRN2", use_seq_codegen=True)

    xT = nc.dram_tensor("xT", [FPAD, NPAD], bf16, kind="ExternalInput")
    WWd = nc.dram_tensor("WWd", [FPAD, 128], bf16, kind="ExternalInput")
    adjT = nc.dram_tensor("adjT", [NPAD, IPAD], bf16, kind="ExternalInput")
    aa = nc.dram_tensor("aa", [128, 2, OUTS], f32, kind="ExternalInput")
    wgb = nc.dram_tensor("wgb", [128, 2, OUTS], f32, kind="ExternalInput")
    cst = nc.dram_tensor("cst", [128, 4], f32, kind="ExternalInput")
    out = nc.dram_tensor("out", [N, OUTS], f32, kind="ExternalOutput")

    with tile.TileContext(nc) as tc:
        with (
            tc.tile_pool(name="cpool", bufs=1) as cp,
            tc.tile_pool(name="big", bufs=1) as bigp,
            tc.tile_pool(name="tmp", bufs=2) as tp,
            tc.tile_pool(name="maskp", bufs=3) as mp,
            tc.tile_pool(name="uep", bufs=3) as uep,
            tc.tile_pool(name="dram", bufs=1, space="DRAM") as dp,
        ):
            # ---- constants ----
            aa_s = cp.tile([128, 2, OUTS], f32)
            nc.sync.dma_start(aa_s, aa[:])
            wgb_s = cp.tile([128, 2, OUTS], f32)
            nc.sync.dma_start(wgb_s, wgb[:])
            cst_s = cp.tile([128, 4], f32)
            nc.sync.dma_start(cst_s, cst[:])
            ident = cp.tile([128, 128], f32)
            make_identity(nc, ident)

            es_d = dp.tile([IPAD], f32)

            # ---- resident data ----
            xt_all = bigp.tile([128, FCH, NPAD], bf16)
            for k in range(FCH):
                nc.sync.dma_start(xt_all[:, k, :], xT[128 * k:128 * (k + 1), :])
            ww_all = bigp.tile([128, FCH, 128], bf16)
            for k in range(FCH):
                nc.sync.dma_start(ww_all[:, k, :], WWd[128 * k:128 * (k + 1), :])

            hid_all = bigp.tile([128, NB, OUTS], f32)
            h0_all = bigp.tile([128, NB, OUTS], f32)
            h_all = bigp.tile([128, NB, OUTS + 1], bf16)
            scr4 = bigp.tile([128, NB, 2, OUTS], f32)
            gb_all = bigp.tile([128, NB, 2], f32)
            pq_all = bigp.tile([128, NB, 2], f32)
            ee_all = bigp.tile([128, NB, 2], f32)
            BD = bigp.tile([128, 2, NB], f32)
            g_all = bigp.tile([128, NGRP, IG], bf16)

            nc.vector.memset(h_all[:, :, OUTS:OUTS + 1], 1.0)

            # ---- projection: [h0 | hidden] = x @ [W_m | Wc1] (bias folded) ----
            with tc.tile_pool(name="ppsum", bufs=2, space="PSUM") as pp:
                for b in range(NB):
                    hh = pp.tile([128, 128], f32, name="hh", tag="hh")
                    for k in range(FCH):
                        nc.tensor.matmul(
                            hh,
                            lhsT=xt_all[:, k, 128 * b:128 * (b + 1)],
                            rhs=ww_all[:, k, :],
                            start=(k == 0),
                            stop=(k == FCH - 1),
                        )
                    nc.scalar.activation(hid_all[:, b, :], hh[:, OUTS:128], Act.Relu)
                    nc.scalar.copy(h0_all[:, b, :], hh[:, 0:OUTS])

            # ---- conditioner epilogue (batched over all blocks) ----
            sh4 = (128, NB, 2, OUTS)
            nc.vector.tensor_tensor(
                scr4, hid_all.unsqueeze(2).to_broadcast(sh4),
                wgb_s.unsqueeze(1).to_broadcast(sh4), Alu.mult)
            nc.vector.tensor_reduce(gb_all, scr4, axis=mybir.AxisListType.X, op=Alu.add)
            sh3 = (128, NB, 2)
            nc.vector.tensor_tensor(
                gb_all, gb_all, cst_s[:, 0:2].unsqueeze(1).to_broadcast(sh3), Alu.add)
            nc.vector.tensor_tensor(
                scr4, h0_all.unsqueeze(2).to_broadcast(sh4),
                aa_s.unsqueeze(1).to_broadcast(sh4), Alu.mult)
            nc.vector.tensor_reduce(pq_all, scr4, axis=mybir.AxisListType.X, op=Alu.add)
            # es/ed = gamma * (p,q) + beta * (sum a_src, sum a_dst)
            t1 = tp.tile([128, NB, 2], f32, tag="t1")
            nc.vector.tensor_tensor(
                t1, pq_all, gb_all[:, :, 0:1].to_broadcast(sh3), Alu.mult)
            t2 = tp.tile([128, NB, 2], f32, tag="t2")
            nc.vector.tensor_tensor(
                t2, gb_all[:, :, 1:2].to_broadcast(sh3),
                cst_s[:, 2:4].unsqueeze(1).to_broadcast(sh3), Alu.mult)
            nc.vector.tensor_tensor(ee_all, t1, t2, Alu.add)
            # FiLM: h = gamma * h0 + beta   (values, bf16, + ones column)
            shh = (128, NB, OUTS)
            nc.vector.tensor_tensor(
                h_all[:, :, 0:OUTS], h0_all,
                gb_all[:, :, 0:1].to_broadcast(shh), Alu.mult)
            nc.vector.tensor_tensor(
                h_all[:, :, 0:OUTS], h_all[:, :, 0:OUTS],
                gb_all[:, :, 1:2].to_broadcast(shh), Alu.add)
            # B = exp(ed), D = exp(0.2 ed)
            nc.scalar.activation(BD[:, 0, :], ee_all[:, :, 1], Act.Exp)
            nc.scalar.activation(BD[:, 1, :], ee_all[:, :, 1], Act.Exp, scale=0.2)
            # es -> DRAM (linear), via PE transpose so the DMA is contiguous
            with tc.tile_pool(name="espsum", bufs=1, space="PSUM") as esp:
                est = esp.tile([NB, 128], f32)
                nc.tensor.transpose(est, ee_all[:, :, 0], ident)
                es_sb = tp.tile([NB, 128], f32, tag="es_sb")
                nc.vector.tensor_copy(es_sb, est)
            nc.gpsimd.dma_start(
                es_d[0:NPAD].rearrange("(b p) -> b p", p=128), es_sb)
            zt = cp.tile([1, IPAD - NPAD], f32)
            nc.vector.memset(zt, 0.0)
            nc.gpsimd.dma_start(
                es_d[NPAD:IPAD].rearrange("(a n) -> a n", a=1), zt)
            # G = exp(-0.8 es), broadcast across partitions
            for g in range(NGRP):
                esb = tp.tile([128, IG], f32, tag="esb")
                nc.sync.dma_start(esb, es_d[IG * g:IG * (g + 1)].partition_broadcast(128))
                nc.scalar.activation(g_all[:, g, :], esb, Act.Exp, scale=-0.8)

            # ---- attention ----
            with tc.tile_pool(name="apsum", bufs=1, space="PSUM") as app:
                po = [app.tile([65, IG], f32, name=f"po{g}", tag=f"po{g}")
                      for g in range(NGRP)]
                for j in range(NB):
                    mt = mp.tile([128, IPAD], bf16, tag="mt")
                    nc.sync.dma_start(mt, adjT[128 * j:128 * (j + 1), :])
                    for g in range(NGRP):
                        U = uep.tile([128, IG], bf16, tag="U")
                        nc.vector.tensor_scalar(
                            U, g_all[:, g, :], BD[:, 1, j:j + 1], BD[:, 0, j:j + 1],
                            Alu.mult, Alu.max)
                        E = uep.tile([128, IG], bf16, tag="E")
                        nc.vector.tensor_tensor(E, U, mt[:, IG * g:IG * (g + 1)], Alu.mult)
                        for s in range(NSUB):
                            nc.tensor.matmul(
                                po[g][:, 512 * s:512 * (s + 1)],
                                lhsT=h_all[:, j, :],
                                rhs=E[:, 512 * s:512 * (s + 1)],
                                start=(j == 0),
                                stop=(j == NB - 1),
                            )

                # ---- normalize + transpose + store ----
                with tc.tile_pool(name="tpsum", bufs=2, space="PSUM") as tpp:
                    for g in range(NGRP):
                        oT = tp.tile([128, IG], f32, tag="oT")
                        nc.vector.memset(oT[64:128, :], 0.0)
                        nc.scalar.copy(oT[0:65, :], po[g])
                        for t in range(IG // 128):
                            i0 = IG * g + 128 * t
                            if i0 >= N:
                                break
                            v = min(128, N - i0)
                            pt = tpp.tile([128, 128], f32, tag="pt")
                            nc.tensor.transpose(pt, oT[:, 128 * t:128 * (t + 1)], ident)
                            r = tp.tile([128, 1], f32, tag="r")
                            nc.vector.reciprocal(r[:v], pt[:v, OUTS:OUTS + 1])
                            os_ = tp.tile([128, OUTS], f32, tag="os")
                            nc.vector.tensor_scalar_mul(os_[:v], pt[:v, 0:OUTS], r[:v])
                            nc.sync.dma_start(out[i0:i0 + v, :], os_[:v])

    nc.finalize()
    _split_multi_waits(nc, mybir)
    return nc


def _split_multi_waits(nc, mybir):
    """This toolchain's walrus accepts at most one sync wait per HW-decoded
    instruction; hoist extra waits onto standalone EventSemaphore ops on the
    same engine (engines execute their stream in order, so semantics hold)."""
    uid = [0]
    for f in nc.m.functions:
        for bb in f.blocks:
            insts = list(bb.instructions)
            out = []
            changed = False
            for ins in insts:
                si = ins.sync_info
                waits = list(si.on_wait) if si is not None and si.on_wait else []
                if len(waits) > 1:
                    changed = True
                    for w in waits[:-1]:
                        uid[0] += 1
                        ev = mybir.InstEventSemaphore(
                            name=f"splitw_{uid[0]}", ins=[], outs=[])
                        ev.engine = ins.engine
                        ev.sync_info = mybir.SyncInfo(on_wait=[w], on_update=[])
                        out.append(ev)
                    si.on_wait = [waits[-1]]
                out.append(ins)
            if changed:
                bb.instructions = out


def _prep_in_maps(x, adj, W, a_src, a_dst, Wc1, bc1, Wc2, bc2):
    bf = ml_dtypes.bfloat16
    xT_h = np.zeros((FPAD, NPAD), dtype=bf)
    xT_h[:INS, :N] = x.T.astype(bf)
    xT_h[INS, :N] = 1.0  # ones row folds the conditioner bias into the matmul

    adjT_h = np.zeros((NPAD, IPAD), dtype=bf)
    adjT_h[:N, :N] = adj.T.astype(bf)

    in_maps = []
    for m in range(M):
        WW_h = np.zeros((FPAD, 128), dtype=bf)
        WW_h[:INS, 0:OUTS] = W[m].astype(bf)
        WW_h[:INS, OUTS:128] = Wc1.astype(bf)
        WW_h[INS, OUTS:128] = bc1.astype(bf)

        aa_h = np.empty((128, 2, OUTS), dtype=np.float32)
        aa_h[:, 0, :] = a_src[m][None, :]
        aa_h[:, 1, :] = a_dst[m][None, :]

        wgb_h = np.empty((128, 2, OUTS), dtype=np.float32)
        wgb_h[:, 0, :] = Wc2[:, m][None, :]
        wgb_h[:, 1, :] = Wc2[:, M + m][None, :]

        cst_h = np.empty((128, 4), dtype=np.float32)
        cst_h[:, 0] = bc2[m]
        cst_h[:, 1] = bc2[M + m]
        cst_h[:, 2] = float(np.sum(a_src[m], dtype=np.float64))
        cst_h[:, 3] = float(np.sum(a_dst[m], dtype=np.float64))

        in_maps.append({
            "xT": xT_h, "WWd": WW_h, "adjT": adjT_h,
            "aa": aa_h, "wgb": wgb_h, "cst": cst_h,
        })
    return in_maps


def kernel(x, adj, W, a_src, a_dst, Wc1, bc1, Wc2, bc2, _profile=False):
    x = np.asarray(x, dtype=np.float32)
    adj = np.asarray(adj)
    W = np.asarray(W, dtype=np.float32)
    a_src = np.asarray(a_src, dtype=np.float32)
    a_dst = np.asarray(a_dst, dtype=np.float32)
    Wc1 = np.asarray(Wc1, dtype=np.float32)
    bc1 = np.asarray(bc1, dtype=np.float32)
    Wc2 = np.asarray(Wc2, dtype=np.float32)
    bc2 = np.asarray(bc2, dtype=np.float32)

    if "nc" not in _CACHE:
        _CACHE["nc"] = _build_nc()
    nc = _CACHE["nc"]

    from concourse.bass_utils import run_bass_kernel_spmd

    in_maps = _prep_in_maps(x, adj, W, a_src, a_dst, Wc1, bc1, Wc2, bc2)
    res = run_bass_kernel_spmd(
        nc, in_maps, core_ids=list(range(M)), trace=_profile,
    )
    full = np.empty((N, M * OUTS), dtype=np.float32)
    for m in range(M):
        full[:, OUTS * m:OUTS * (m + 1)] = res.results[m]["out"]
    if _profile:
        return full, res
    return full


# revision 12
# speedup vs baseline: 1.0949x; 1.0949x over previous
"""ConditionalAttentionLayer (gnn_message_passing) Trainium2 kernel.

Sharding: one attention mechanism (head) per NeuronCore, 8 cores.
Each core computes its head's projection h_m = FiLM(x @ W_m), GAT scores,
masked softmax and out_m = attn_m @ h_m over all N=2708 nodes; the host
concatenates the 8 [N, 64] slices into [N, 512].

Math trick used on-device: with s_ij = es_i + ed_j,
  exp(leakyrelu(s)) = max(exp(s), exp(0.2 s))          (exp is monotone)
and softmax over j is invariant to any per-i scale, so dividing by
exp(es_i) gives
  E_ij = adj_ji * max(G_i * D_j, B_j)
with G = exp(-0.8 es), B = exp(ed), D = exp(0.2 ed).  Each [128, 1536]
attention tile is then just one dual-op tensor_scalar (mult+max) and one
masked multiply on DVE — no per-tile transcendentals.  Row sums for the
softmax come free from a ones-column appended to the value matrix in the
PE matmul (output is computed transposed, [65, i], then PE-transposed
back and normalized).
"""

import numpy as np
import ml_dtypes

N = 2708
INS = 1433
OUTS = 64
M = 8
HID = 64

FCH = 12            # feature chunks of 128 (1433 + ones row -> 1536)
FPAD = FCH * 128    # 1536
NB = 22             # node blocks of 128
NPAD = NB * 128     # 2816
NGRP = 2            # i groups
IG = 1408           # i group width (2 * 1408 = 2816 covers N exactly)
IPAD = NGRP * IG    # 2816
SUBS = [(0, 512), (512, 512), (1024, 384)]  # sub-matmul slices per i group
PGRP = [range(0, 6), range(6, 12), range(12, 18), range(18, 22)]

_CACHE = {}


def _build_nc():
    import concourse.bass as bass
    import concourse.mybir as mybir
    import concourse.tile as tile
    from concourse.masks import make_identity

    f32 = mybir.dt.float32
    bf16 = mybir.dt.bfloat16
    Alu = mybir.AluOpType
    Act = mybir.ActivationFunctionType

    nc = bass.Bass("TRN2", use_seq_codegen=True)

    xT = nc.dram_tensor("xT", [FPAD, NPAD], bf16, kind="ExternalInput")
    WWd = nc.dram_tensor("WWd", [FPAD, 128], bf16, kind="ExternalInput")
    adjT = nc.dram_tensor("adjT", [NPAD, IPAD], bf16, kind="ExternalInput")
    aa = nc.dram_tensor("aa", [128, 2, OUTS], f32, kind="ExternalInput")
    wgb = nc.dram_tensor("wgb", [128, 2, OUTS], f32, kind="ExternalInput")
    cst = nc.dram_tensor("cst", [128, 4], f32, kind="ExternalInput")
    out = nc.dram_tensor("out", [N, OUTS], f32, kind="ExternalOutput")

    with tile.TileContext(nc) as tc:
        with (
            tc.tile_pool(name="cpool", bufs=1) as cp,
            tc.tile_pool(name="big", bufs=1) as bigp,
            tc.tile_pool(name="tmp", bufs=2) as tp,
            tc.tile_pool(name="maskp", bufs=3) as mp,
            tc.tile_pool(name="uep", bufs=3) as uep,
            tc.tile_pool(name="dram", bufs=1, space="DRAM") as dp,
        ):
            # ---- constants ----
            aa_s = cp.tile([128, 2, OUTS], f32)
            nc.sync.dma_start(aa_s, aa[:])
            wgb_s = cp.tile([128, 2, OUTS], f32)
            nc.sync.dma_start(wgb_s, wgb[:])
            cst_s = cp.tile([128, 4], f32)
            nc.sync.dma_start(cst_s, cst[:])
            ident = cp.tile([128, 128], f32)
            make_identity(nc, ident)

            es_d = dp.tile([IPAD], f32)

            # ---- resident data (small WW first so matmuls can start on the
            # first xT chunk) ----
            ww_all = bigp.tile([128, FCH, 128], bf16)
            for k in range(FCH):
                nc.sync.dma_start(ww_all[:, k, :], WWd[128 * k:128 * (k + 1), :])
            xt_all = bigp.tile([128, FCH, NPAD], bf16)
            for k in range(FCH):
                nc.sync.dma_start(xt_all[:, k, :], xT[128 * k:128 * (k + 1), :])

            hid_all = bigp.tile([128, NB, OUTS], f32)
            h0_all = bigp.tile([128, NB, OUTS], f32)
            h_all = bigp.tile([128, NB, OUTS + 1], bf16)
            scr4 = bigp.tile([128, NB, 2, OUTS], f32)
            gb_all = bigp.tile([128, NB, 2], f32)
            pq_all = bigp.tile([128, NB, 2], f32)
            ee_all = bigp.tile([128, NB, 2], f32)
            BD = bigp.tile([128, 2, NB], f32)
            g_all = bigp.tile([128, NGRP, IG], bf16)

            nc.vector.memset(h_all[:, :, OUTS:OUTS + 1], 1.0)

            # ---- projection: [h0 | hidden] = x @ [W_m | Wc1] (bias folded) ----
            # k-inner over groups of 6 blocks: matmuls start on the first xT
            # chunk and PE stays dense; conditioner reduce per group hides
            # under the next group's matmuls.
            with tc.tile_pool(name="ppsum", bufs=1, space="PSUM") as pp:
                for blocks in PGRP:
                    hhs = {}
                    for t, b in enumerate(blocks):
                        hhs[b] = pp.tile([128, 128], f32, name=f"hh{t}",
                                         tag=f"hh{t}")
                    for k in range(FCH):
                        for b in blocks:
                            nc.tensor.matmul(
                                hhs[b],
                                lhsT=xt_all[:, k, 128 * b:128 * (b + 1)],
                                rhs=ww_all[:, k, :],
                                start=(k == 0),
                                stop=(k == FCH - 1),
                            )
                    for b in blocks:
                        nc.scalar.activation(hid_all[:, b, :], hhs[b][:, OUTS:128],
                                             Act.Relu)
                        nc.scalar.copy(h0_all[:, b, :], hhs[b][:, 0:OUTS])
                    b0, b1 = blocks.start, blocks.stop
                    nbk = b1 - b0
                    sh4g = (128, nbk, 2, OUTS)
                    nc.vector.tensor_tensor(
                        scr4[:, b0:b1],
                        hid_all[:, b0:b1].unsqueeze(2).to_broadcast(sh4g),
                        wgb_s.unsqueeze(1).to_broadcast(sh4g), Alu.mult)
                    nc.vector.tensor_reduce(
                        gb_all[:, b0:b1], scr4[:, b0:b1],
                        axis=mybir.AxisListType.X, op=Alu.add)
                    nc.vector.tensor_tensor(
                        scr4[:, b0:b1],
                        h0_all[:, b0:b1].unsqueeze(2).to_broadcast(sh4g),
                        aa_s.unsqueeze(1).to_broadcast(sh4g), Alu.mult)
                    nc.vector.tensor_reduce(
                        pq_all[:, b0:b1], scr4[:, b0:b1],
                        axis=mybir.AxisListType.X, op=Alu.add)

                # gamma/beta += bc2; es/ed = gamma*(p,q) + beta*(sum a)
                sh3 = (128, NB, 2)
                nc.vector.tensor_tensor(
                    gb_all, gb_all, cst_s[:, 0:2].unsqueeze(1).to_broadcast(sh3),
                    Alu.add)
                t1 = tp.tile([128, NB, 2], f32, tag="t1")
                nc.vector.tensor_tensor(
                    t1, pq_all, gb_all[:, :, 0:1].to_broadcast(sh3), Alu.mult)
                t2 = tp.tile([128, NB, 2], f32, tag="t2")
                nc.vector.tensor_tensor(
                    t2, gb_all[:, :, 1:2].to_broadcast(sh3),
                    cst_s[:, 2:4].unsqueeze(1).to_broadcast(sh3), Alu.mult)
                nc.vector.tensor_tensor(ee_all, t1, t2, Alu.add)
                # kick the serial es chain first: transpose -> DRAM -> G bcast
                est = pp.tile([NB, 128], f32, name="est", tag="hh0")
                nc.tensor.transpose(est, ee_all[:, :, 0], ident)
                es_sb = tp.tile([NB, 128], f32, tag="es_sb")
                nc.vector.tensor_copy(es_sb, est)
                nc.gpsimd.dma_start(
                    es_d[0:NPAD].rearrange("(b p) -> b p", p=128), es_sb)
                for g in range(NGRP):
                    esb = tp.tile([128, IG], f32, tag="esb")
                    nc.sync.dma_start(
                        esb, es_d[IG * g:IG * (g + 1)].partition_broadcast(128))
                    nc.scalar.activation(g_all[:, g, :], esb, Act.Exp, scale=-0.8)
                # B = exp(ed), D = exp(0.2 ed)
                nc.scalar.activation(BD[:, 0, :], ee_all[:, :, 1], Act.Exp)
                nc.scalar.activation(BD[:, 1, :], ee_all[:, :, 1], Act.Exp,
                                     scale=0.2)
                # FiLM: h = gamma * h0 + beta (overlaps the es DRAM roundtrip)
                shh = (128, NB, OUTS)
                nc.vector.tensor_tensor(
                    h_all[:, :, 0:OUTS], h0_all,
                    gb_all[:, :, 0:1].to_broadcast(shh), Alu.mult)
                nc.vector.tensor_tensor(
                    h_all[:, :, 0:OUTS], h_all[:, :, 0:OUTS],
                    gb_all[:, :, 1:2].to_broadcast(shh), Alu.add)

            # ---- attention ----
            with tc.tile_pool(name="apsum", bufs=1, space="PSUM") as app:
                po = [app.tile([65, IG], f32, name=f"po{g}", tag=f"po{g}")
                      for g in range(NGRP)]
                for j in range(NB):
                    mt = mp.tile([128, IPAD], bf16, tag="mt")
                    nc.sync.dma_start(mt, adjT[128 * j:128 * (j + 1), :])
                    for g in range(NGRP):
                        U = uep.tile([128, IG], bf16, tag="U")
                        nc.vector.tensor_scalar(
                            U, g_all[:, g, :], BD[:, 1, j:j + 1], BD[:, 0, j:j + 1],
                            Alu.mult, Alu.max)
                        E = uep.tile([128, IG], bf16, tag="E")
                        nc.vector.tensor_tensor(E, U, mt[:, IG * g:IG * (g + 1)], Alu.mult)
                        for s0, sw in SUBS:
                            nc.tensor.matmul(
                                po[g][:, s0:s0 + sw],
                                lhsT=h_all[:, j, :],
                                rhs=E[:, s0:s0 + sw],
                                start=(j == 0),
                                stop=(j == NB - 1),
                            )

                # ---- normalize + transpose + store ----
                with tc.tile_pool(name="tpsum", bufs=2, space="PSUM") as tpp:
                    for g in range(NGRP):
                        oT = tp.tile([128, IG], f32, tag="oT")
                        nc.vector.memset(oT[64:128, :], 0.0)
                        nc.scalar.copy(oT[0:65, :], po[g])
                        for t in range(IG // 128):
                            i0 = IG * g + 128 * t
                            if i0 >= N:
                                break
                            v = min(128, N - i0)
                            pt = tpp.tile([128, 128], f32, tag="pt")
                            nc.tensor.transpose(pt, oT[:, 128 * t:128 * (t + 1)], ident)
                            r = tp.tile([128, 1], f32, tag="r", bufs=4)
                            nc.vector.reciprocal(r[:v], pt[:v, OUTS:OUTS + 1])
                            os_ = tp.tile([128, OUTS], f32, tag="os", bufs=4)
                            nc.vector.tensor_scalar_mul(os_[:v], pt[:v, 0:OUTS], r[:v])
                            nc.sync.dma_start(out[i0:i0 + v, :], os_[:v])

    nc.finalize()
    _split_multi_waits(nc, mybir)
    return nc


def _split_multi_waits(nc, mybir):
    """This toolchain's walrus accepts at most one sync wait per HW-decoded
    instruction; hoist extra waits onto standalone EventSemaphore ops on the
    same engine (engines execute their stream in order, so semantics hold)."""
    uid = [0]
    for f in nc.m.functions:
        for bb in f.blocks:
            insts = list(bb.instructions)
            out = []
            changed = False
            for ins in insts:
                si = ins.sync_info
                waits = list(si.on_wait) if si is not None and si.on_wait else []
                if len(waits) > 1:
                    changed = True
                    for w in waits[:-1]:
                        uid[0] += 1
                        ev = mybir.InstEventSemaphore(
                            name=f"splitw_{uid[0]}", ins=[], outs=[])
                        ev.engine = ins.engine
                        ev.sync_info = mybir.SyncInfo(on_wait=[w], on_update=[])
                        out.append(ev)
                    si.on_wait = [waits[-1]]
                out.append(ins)
            if changed:
                bb.instructions = out


def _prep_in_maps(x, adj, W, a_src, a_dst, Wc1, bc1, Wc2, bc2):
    bf = ml_dtypes.bfloat16
    xT_h = np.zeros((FPAD, NPAD), dtype=bf)
    xT_h[:INS, :N] = x.T.astype(bf)
    xT_h[INS, :N] = 1.0  # ones row folds the conditioner bias into the matmul

    adjT_h = np.zeros((NPAD, IPAD), dtype=bf)
    adjT_h[:N, :N] = adj.T.astype(bf)

    in_maps = []
    for m in range(M):
        WW_h = np.zeros((FPAD, 128), dtype=bf)
        WW_h[:INS, 0:OUTS] = W[m].astype(bf)
        WW_h[:INS, OUTS:128] = Wc1.astype(bf)
        WW_h[INS, OUTS:128] = bc1.astype(bf)

        aa_h = np.empty((128, 2, OUTS), dtype=np.float32)
        aa_h[:, 0, :] = a_src[m][None, :]
        aa_h[:, 1, :] = a_dst[m][None, :]

        wgb_h = np.empty((128, 2, OUTS), dtype=np.float32)
        wgb_h[:, 0, :] = Wc2[:, m][None, :]
        wgb_h[:, 1, :] = Wc2[:, M + m][None, :]

        cst_h = np.empty((128, 4), dtype=np.float32)
        cst_h[:, 0] = bc2[m]
        cst_h[:, 1] = bc2[M + m]
        cst_h[:, 2] = float(np.sum(a_src[m], dtype=np.float64))
        cst_h[:, 3] = float(np.sum(a_dst[m], dtype=np.float64))

        in_maps.append({
            "xT": xT_h, "WWd": WW_h, "adjT": adjT_h,
            "aa": aa_h, "wgb": wgb_h, "cst": cst_h,
        })
    return in_maps


def kernel(x, adj, W, a_src, a_dst, Wc1, bc1, Wc2, bc2, _profile=False):
    x = np.asarray(x, dtype=np.float32)
    adj = np.asarray(adj)
    W = np.asarray(W, dtype=np.float32)
    a_src = np.asarray(a_src, dtype=np.float32)
    a_dst = np.asarray(a_dst, dtype=np.float32)
    Wc1 = np.asarray(Wc1, dtype=np.float32)
    bc1 = np.asarray(bc1, dtype=np.float32)
    Wc2 = np.asarray(Wc2, dtype=np.float32)
    bc2 = np.asarray(bc2, dtype=np.float32)

    if "nc" not in _CACHE:
        _CACHE["nc"] = _build_nc()
    nc = _CACHE["nc"]

    from concourse.bass_utils import run_bass_kernel_spmd

    in_maps = _prep_in_maps(x, adj, W, a_src, a_dst, Wc1, bc1, Wc2, bc2)
    res = run_bass_kernel_spmd(
        nc, in_maps, core_ids=list(range(M)), trace=_profile,
    )
    full = np.empty((N, M * OUTS), dtype=np.float32)
    for m in range(M):
        full[:, OUTS * m:OUTS * (m + 1)] = res.results[m]["out"]
    if _profile:
        return full, res
    return full
